# revision 20
# baseline (speedup 1.0000x reference)
"""BCQConv1D TRN2 kernel: out[b,s,o] = x[b,s,:] @ W[o,:]^T + bias[o],
W[o, g*A+a] = sum_qb alpha[o,g,qb] * binary[o,g,a,qb].

Sharding: column-parallel — alpha/binary/bias split along out_features
across the 8 NeuronCores, x replicated (each core computes the full
batch for its 512 output features).

Per core: reconstruct the W^T shard on device, keep it resident in SBUF
as bf16, then stream x^T tiles and run N=512 bf16 matmuls accumulating
over K=4096 in PSUM. Bias is broadcast once via a K=1 ones-matmul and
folded into the PSUM->SBUF output add on the DVE.

Reconstruction: binary ships as fp8e4 (+-1 is exact) laid out
[O, G, QB, A]; alpha ships [O, QB, G]. The per-bit-plane scaling runs as
large DVE tensor_tensor ops ([128, NGB*A] at a time) with alpha
broadcast along the A axis via step-0 APs, planes summed on DVE, then
one PE transpose per (g, o-tile) into a dedicated PSUM pool and a
single [128, O_SH] copy per group into the resident W^T tiles.

Host side only slices/relayouts/casts inputs (x is passed
transposed/tiled [128, KT, BS] so every DMA line is contiguous).
"""

import numpy as np

import concourse.bass as bass
import concourse.tile as tile
from concourse import bacc, mybir
from concourse.bass_utils import run_bass_kernel_spmd
from concourse.masks import make_identity

# Problem shape (hardcoded per contest contract)
B, S, I, O = 4, 2048, 4096, 4096
G, A, QB = 32, 128, 3  # n_groups, group_size, qbits; G*A == I
BS = B * S  # 8192
P = 128
KT = I // P  # 32 k-tiles (== groups: i = g*A + a, A == P)

# Sharding
N_CORES = 8
O_WAYS = 8
BS_WAYS = 1
O_SH = O // O_WAYS  # per-core out features
BS_SH = BS // BS_WAYS  # per-core batch rows
NFREE = 512  # matmul moving free dim (one PSUM bank of fp32)
NB = O_SH // NFREE  # o-blocks per core
BCHUNK = 512  # bs columns fetched per x DMA
NSUB = BCHUNK // P  # matmul chains per x chunk
NGB = 8  # groups per recon DVE op block

F32 = mybir.dt.float32
F32R = mybir.dt.float32r
BF16 = mybir.dt.bfloat16
F8E4 = mybir.dt.float8e4

OT = O_SH // P  # o-tiles for recon



def build_nc():
    nc = bacc.Bacc(target_bir_lowering=False)
    xt_d = nc.declare_dram_parameter("xt", [P, KT, BS_SH], BF16, isOutput=False)
    alpha_d = nc.declare_dram_parameter("alpha", [O_SH, QB, G], F32, isOutput=False)
    binary_d = nc.declare_dram_parameter(
        "binary", [O_SH, G, QB, A], F8E4, isOutput=False
    )
    bias_d = nc.declare_dram_parameter("bias", [O_SH], F32, isOutput=False)
    out_d = nc.declare_dram_parameter("out", [BS_SH, O_SH], BF16, isOutput=True)

    add = mybir.AluOpType.add
    mult = mybir.AluOpType.mult

    with tile.TileContext(nc) as tc:
        with (
            tc.tile_pool(name="const", bufs=1) as cpool,
            tc.tile_pool(name="wt", bufs=1) as wtpool,
            tc.tile_pool(name="bin16", bufs=2) as bin16p,
            tc.tile_pool(name="wacc", bufs=2) as wacc,
            tc.tile_pool(name="xp", bufs=48) as xp,
            tc.tile_pool(name="op", bufs=8) as op,
            tc.tile_pool(name="rpsum", bufs=2, space="PSUM") as rp,
            tc.tile_pool(name="psum", bufs=6, space="PSUM") as pp,
        ):
            # --- constants ---
            ident = cpool.tile([P, P], BF16, name="ident")
            make_identity(nc, ident)
            ones = cpool.tile([1, P], F32, name="ones")
            nc.vector.memset(ones, 1.0)
            bias_row = cpool.tile([1, O_SH], F32, name="bias_row")
            nc.sync.dma_start(out=bias_row, in_=bias_d.ap().unsqueeze(0))
            bias_bc = cpool.tile([P, O_SH], F32, name="bias_bc")
            for j in range(NB):
                pbt = pp.tile([P, NFREE], F32, tag="ps", name=f"psb{j}")
                nc.tensor.matmul(
                    pbt, ones, bias_row[:, j * NFREE : (j + 1) * NFREE],
                    start=True, stop=True,
                )
                nc.vector.tensor_copy(
                    out=bias_bc[:, j * NFREE : (j + 1) * NFREE], in_=pbt
                )

            # --- recon inputs: alpha resident per o-tile (fast sync ring) ---
            alpha_sb = []
            for ot in range(OT):
                at = cpool.tile([P, QB, G], F32, name=f"alpha{ot}")
                nc.sync.dma_start(out=at, in_=alpha_d.ap()[ot * P : (ot + 1) * P])
                alpha_sb.append(at)

            # --- W^T shard, resident, one tile per k-tile (== group) ---
            wt_tiles = [
                wtpool.tile([P, O_SH], BF16, tag=f"wt{k}", name=f"wt{k}")
                for k in range(KT)
            ]

            # --- reconstruction ---
            # Per (gp, ot): scale each bit plane with alpha broadcast along A
            # (DVE/GpSimd tensor_tensor), then sum the planes on the PE with
            # regular accumulating matmuls against the identity (which also
            # transposes [o,a] -> [a,o]); ACT copies PSUM -> resident W^T.
            for gp in range(G // NGB):
                g0 = gp * NGB
                planes = []  # [ot][b] alpha-scaled bit planes
                for ot in range(OT):
                    at = alpha_sb[ot]
                    b16 = bin16p.tile([P, NGB, QB, A], F8E4, tag=f"b16_{ot}")
                    nc.gpsimd.dma_start(
                        out=b16,
                        in_=binary_d.ap()[ot * P : (ot + 1) * P, g0 : g0 + NGB],
                    )

                    def abc(b):
                        return at[:, b, g0 : g0 + NGB, None].to_broadcast(
                            (P, NGB, A)
                        )

                    sc0 = wacc.tile([P, NGB, A], BF16, tag=f"sc{ot}_0")
                    sc1 = wacc.tile([P, NGB, A], BF16, tag=f"sc{ot}_1")
                    sc2 = wacc.tile([P, NGB, A], BF16, tag=f"sc{ot}_2")
                    nc.vector.tensor_tensor(
                        out=sc0, in0=b16[:, :, 0, :], in1=abc(0), op=mult
                    )
                    nc.gpsimd.tensor_tensor(
                        out=sc1, in0=b16[:, :, 1, :], in1=abc(1), op=mult
                    )
                    nc.vector.tensor_tensor(
                        out=sc2, in0=b16[:, :, 2, :], in1=abc(2), op=mult
                    )
                    nc.gpsimd.tensor_tensor(out=sc0, in0=sc0, in1=sc1, op=add)
                    nc.vector.tensor_tensor(out=sc0, in0=sc0, in1=sc2, op=add)
                    planes.append(sc0)
                for gl in range(NGB):
                    g = g0 + gl
                    pt = rp.tile([P, O_SH], BF16, tag="rps", name=f"rps{g}")
                    for ot in range(OT):
                        nc.tensor.transpose(
                            pt[:, ot * P : (ot + 1) * P], planes[ot][:, gl, :], ident
                        )
                    nc.scalar.copy(out=wt_tiles[g], in_=pt)

            # --- main matmul: out[bs, o] = x^T.T @ W^T (+bias) ---
            # s-major: each 128-row sub-chain runs its full k-loop against a
            # single PSUM bank, so chunk boundaries need only one free bank
            # at a time and drains overlap the next chain's accumulation.
            n_chunks = BS_SH // BCHUNK
            for c in range(n_chunks):
                xts = []
                for k in range(KT):
                    xt_t = xp.tile([P, BCHUNK], BF16, tag="xt")
                    dma_eng = nc.sync if k % 2 == 0 else nc.scalar
                    dma_eng.dma_start(
                        out=xt_t,
                        in_=xt_d.ap()[:, k, c * BCHUNK : (c + 1) * BCHUNK],
                    )
                    xts.append(xt_t)
                for s in range(NSUB):
                    for j in range(NB):
                        ps = pp.tile([P, NFREE], F32, tag="ps", name=f"mm{c}_{s}_{j}")
                        for k in range(KT):
                            nc.tensor.matmul(
                                ps,
                                xts[k][:, s * P : (s + 1) * P],
                                wt_tiles[k][:, j * NFREE : (j + 1) * NFREE],
                                start=(k == 0),
                                stop=(k == KT - 1),
                            )
                        os_t = op.tile([P, NFREE], BF16, tag="os")
                        nc.vector.tensor_tensor(
                            out=os_t,
                            in0=ps,
                            in1=bias_bc[:, j * NFREE : (j + 1) * NFREE],
                            op=add,
                        )
                        dma_eng = nc.sync if s % 2 == 0 else nc.scalar
                        dma_eng.dma_start(
                            out=out_d.ap()[
                                c * BCHUNK + s * P : c * BCHUNK + (s + 1) * P,
                                j * NFREE : (j + 1) * NFREE,
                            ],
                            in_=os_t,
                        )

    if not nc.is_finalized():
        nc.finalize()
    return nc


def shard_inputs(x, alpha, bias, binary):
    """Host-side slicing/relayout/dtype-cast only. Returns per-core input maps."""
    bf16 = mybir.dt.np(BF16)
    f8 = mybir.dt.np(F8E4)
    x2 = np.ascontiguousarray(x).reshape(BS, I).astype(bf16)
    # xtp[p, k, s] = x2[s, k*P + p]  -> every DMA line is bs-contiguous
    xtp = np.ascontiguousarray(x2.T.reshape(KT, P, BS).transpose(1, 0, 2))
    # alpha [O, G, QB] -> [O, QB, G] so the broadcast AP walks g contiguously
    alpha_t = np.ascontiguousarray(alpha.transpose(0, 2, 1))
    # binary [O, G, A, QB] -> [O, G, QB, A] in bf16 (+-1 exact)
    binary_t = np.ascontiguousarray(binary.transpose(0, 1, 3, 2)).astype(f8)
    bias = np.ascontiguousarray(bias)

    in_maps = []
    for c in range(N_CORES):
        oc, bc = divmod(c, BS_WAYS)
        osl = slice(oc * O_SH, (oc + 1) * O_SH)
        if BS_WAYS == 1:
            xc = xtp
        else:
            xc = np.ascontiguousarray(xtp[:, :, bc * BS_SH : (bc + 1) * BS_SH])
        in_maps.append(
            {
                "xt": xc,
                "alpha": alpha_t[osl],
                "binary": binary_t[osl],
                "bias": bias[osl],
            }
        )
    return in_maps


def assemble_output(results):
    out = np.empty((BS, O), dtype=np.float32)
    for c in range(N_CORES):
        oc, bc = divmod(c, BS_WAYS)
        out[
            bc * BS_SH : (bc + 1) * BS_SH, oc * O_SH : (oc + 1) * O_SH
        ] = results[c]["out"].astype(np.float32)
    return out.reshape(B, S, O)


_NC_CACHE = None


def kernel(x, alpha, bias, binary):
    global _NC_CACHE
    if _NC_CACHE is None:
        _NC_CACHE = build_nc()
    nc = _NC_CACHE
    in_maps = shard_inputs(
        np.asarray(x, dtype=np.float32),
        np.asarray(alpha, dtype=np.float32),
        np.asarray(bias, dtype=np.float32),
        np.asarray(binary, dtype=np.float32),
    )
    res = run_bass_kernel_spmd(nc, in_maps, list(range(N_CORES)))
    return assemble_output(res.results)


# revision 21
# speedup vs baseline: 1.0281x; 1.0281x over previous
"""BCQConv1D TRN2 kernel: out[b,s,o] = x[b,s,:] @ W[o,:]^T + bias[o],
W[o, g*A+a] = sum_qb alpha[o,g,qb] * binary[o,g,a,qb].

Sharding: column-parallel - alpha/binary/bias split along out_features
across the 8 NeuronCores, x replicated (each core computes the full
batch for its 512 output features).

Per core: reconstruct the W^T shard on device, keep it resident in SBUF
as bf16, then stream x^T tiles and run N=512 bf16 matmuls accumulating
over K=4096 in PSUM (s-major: one PSUM bank per 128-row output chain).
Bias is broadcast once via a K=1 ones-matmul and folded into the
PSUM->SBUF output add on the DVE. Output ships back as bf16.

Reconstruction: per (group-block, o-tile) the three bit planes are
scaled by alpha broadcast along A (step-0 AP) as [128, 8*128]
tensor_tensor ops split across DVE and GpSimd, summed on DVE, PE
transposes each [o,a] block into a dedicated 2-bank PSUM staging tile,
one [128, 512] copy per group lands the resident W^T tile.

Host side only slices/relayouts/casts inputs (x is passed
transposed/tiled [128, KT, BS] so every DMA line is contiguous).
"""

import numpy as np

import concourse.bass as bass
import concourse.tile as tile
from concourse import bacc, mybir
from concourse.bass_utils import run_bass_kernel_spmd
from concourse.masks import make_identity

# Problem shape (hardcoded per contest contract)
B, S, I, O = 4, 2048, 4096, 4096
G, A, QB = 32, 128, 3  # n_groups, group_size, qbits; G*A == I
BS = B * S  # 8192
P = 128
KT = I // P  # 32 k-tiles (== groups: i = g*A + a, A == P)

# Sharding
N_CORES = 8
O_WAYS = 8
BS_WAYS = 1
O_SH = O // O_WAYS  # per-core out features
BS_SH = BS // BS_WAYS  # per-core batch rows
NFREE = 512  # matmul moving free dim (one PSUM bank of fp32)
NB = O_SH // NFREE  # o-blocks per core
BCHUNK = 512  # bs columns fetched per x DMA
NSUB = BCHUNK // P  # matmul chains per x chunk
NGB = 8  # groups per recon DVE op block

F32 = mybir.dt.float32
BF16 = mybir.dt.bfloat16
F8E4 = mybir.dt.float8e4

OT = O_SH // P  # o-tiles for recon

BIN_FP8 = False  # ship binary as fp8e4 (half upload) vs bf16


def build_nc():
    nc = bacc.Bacc(target_bir_lowering=False)
    bin_dt = F8E4 if BIN_FP8 else BF16
    xt_d = nc.declare_dram_parameter("xt", [P, KT, BS_SH], BF16, isOutput=False)
    alpha_d = nc.declare_dram_parameter("alpha", [O_SH, G, QB], BF16, isOutput=False)
    binary_d = nc.declare_dram_parameter(
        "binary", [O_SH, G, QB, A], bin_dt, isOutput=False
    )
    bias_d = nc.declare_dram_parameter("bias", [O_SH], F32, isOutput=False)
    out_d = nc.declare_dram_parameter("out", [BS_SH, O_SH], BF16, isOutput=True)

    add = mybir.AluOpType.add
    mult = mybir.AluOpType.mult

    with tile.TileContext(nc) as tc:
        with (
            tc.tile_pool(name="const", bufs=1) as cpool,
            tc.tile_pool(name="wt", bufs=1) as wtpool,
            tc.tile_pool(name="bin16", bufs=2) as bin16p,
            tc.tile_pool(name="wacc", bufs=2) as wacc,
            tc.tile_pool(name="xp", bufs=48) as xp,
            tc.tile_pool(name="op", bufs=8) as op,
            tc.tile_pool(name="rpsum", bufs=2, space="PSUM") as rp,
            tc.tile_pool(name="psum", bufs=6, space="PSUM") as pp,
        ):
            # --- constants ---
            ident = cpool.tile([P, P], BF16, name="ident")
            make_identity(nc, ident)
            ones = cpool.tile([1, P], F32, name="ones")
            nc.vector.memset(ones, 1.0)
            bias_row = cpool.tile([1, O_SH], F32, name="bias_row")
            nc.sync.dma_start(out=bias_row, in_=bias_d.ap().unsqueeze(0))
            bias_bc = cpool.tile([P, O_SH], F32, name="bias_bc")
            for j in range(NB):
                pbt = pp.tile([P, NFREE], F32, tag="ps", name=f"psb{j}")
                nc.tensor.matmul(
                    pbt, ones, bias_row[:, j * NFREE : (j + 1) * NFREE],
                    start=True, stop=True,
                )
                nc.vector.tensor_copy(
                    out=bias_bc[:, j * NFREE : (j + 1) * NFREE], in_=pbt
                )

            # --- recon inputs: alpha resident per o-tile (fast sync ring) ---
            alpha_sb = []
            for ot in range(OT):
                at = cpool.tile([P, G, QB], BF16, name=f"alpha{ot}")
                nc.sync.dma_start(out=at, in_=alpha_d.ap()[ot * P : (ot + 1) * P])
                alpha_sb.append(at)

            # --- W^T shard, resident, one tile per k-tile (== group) ---
            wt_tiles = [
                wtpool.tile([P, O_SH], BF16, tag=f"wt{k}", name=f"wt{k}")
                for k in range(KT)
            ]

            # --- reconstruction ---
            for gp in range(G // NGB):
                g0 = gp * NGB
                planes = []
                for ot in range(OT):
                    at = alpha_sb[ot]
                    b16 = bin16p.tile([P, NGB, QB, A], bin_dt, tag=f"b16_{ot}")
                    nc.gpsimd.dma_start(
                        out=b16,
                        in_=binary_d.ap()[ot * P : (ot + 1) * P, g0 : g0 + NGB],
                    )

                    def abc(b):
                        return at[:, g0 : g0 + NGB, b, None].to_broadcast(
                            (P, NGB, A)
                        )

                    acc = wacc.tile([P, NGB, A], BF16, tag=f"sc{ot}_0")
                    tmp = wacc.tile([P, NGB, A], BF16, tag=f"sc{ot}_1")
                    nc.vector.tensor_tensor(
                        out=acc, in0=b16[:, :, 0, :], in1=abc(0), op=mult
                    )
                    nc.gpsimd.tensor_tensor(
                        out=tmp, in0=b16[:, :, 1, :], in1=abc(1), op=mult
                    )
                    nc.vector.tensor_tensor(out=acc, in0=acc, in1=tmp, op=add)
                    tmp2 = wacc.tile([P, NGB, A], BF16, tag=f"sc{ot}_1")
                    nc.vector.tensor_tensor(
                        out=tmp2, in0=b16[:, :, 2, :], in1=abc(2), op=mult
                    )
                    nc.vector.tensor_tensor(out=acc, in0=acc, in1=tmp2, op=add)
                    planes.append(acc)
                for gl in range(NGB):
                    g = g0 + gl
                    pt = rp.tile([P, O_SH], BF16, tag="rps", name=f"rps{g}")
                    for ot in range(OT):
                        nc.tensor.transpose(
                            pt[:, ot * P : (ot + 1) * P], planes[ot][:, gl, :], ident
                        )
                    nc.vector.tensor_copy(out=wt_tiles[g], in_=pt)

            # --- main matmul: out[bs, o] = x^T.T @ W^T (+bias) ---
            # s-major: each 128-row sub-chain runs its full k-loop against a
            # single PSUM bank, so chunk boundaries need only one free bank
            # at a time and drains overlap the next chain's accumulation.
            n_chunks = BS_SH // BCHUNK
            for c in range(n_chunks):
                xts = []
                for k in range(KT):
                    xt_t = xp.tile([P, BCHUNK], BF16, tag="xt")
                    dma_eng = nc.sync if k % 2 == 0 else nc.scalar
                    dma_eng.dma_start(
                        out=xt_t,
                        in_=xt_d.ap()[:, k, c * BCHUNK : (c + 1) * BCHUNK],
                    )
                    xts.append(xt_t)
                for s in range(NSUB):
                    for j in range(NB):
                        ps = pp.tile([P, NFREE], F32, tag="ps", name=f"mm{c}_{s}_{j}")
                        for k in range(KT):
                            nc.tensor.matmul(
                                ps,
                                xts[k][:, s * P : (s + 1) * P],
                                wt_tiles[k][:, j * NFREE : (j + 1) * NFREE],
                                start=(k == 0),
                                stop=(k == KT - 1),
                            )
                        os_t = op.tile([P, NFREE], BF16, tag="os")
                        nc.vector.tensor_tensor(
                            out=os_t,
                            in0=ps,
                            in1=bias_bc[:, j * NFREE : (j + 1) * NFREE],
                            op=add,
                        )
                        dma_eng = nc.sync if s % 2 == 0 else nc.scalar
                        dma_eng.dma_start(
                            out=out_d.ap()[
                                c * BCHUNK + s * P : c * BCHUNK + (s + 1) * P,
                                j * NFREE : (j + 1) * NFREE,
                            ],
                            in_=os_t,
                        )

    if not nc.is_finalized():
        nc.finalize()
    return nc


def shard_inputs(x, alpha, bias, binary):
    """Host-side slicing/relayout/dtype-cast only. Returns per-core input maps."""
    bf16 = mybir.dt.np(BF16)
    f8 = mybir.dt.np(F8E4)
    bin_np = f8 if BIN_FP8 else bf16
    x2 = np.ascontiguousarray(x).reshape(BS, I).astype(bf16)
    # xtp[p, k, s] = x2[s, k*P + p]  -> every DMA line is bs-contiguous
    xtp = np.ascontiguousarray(x2.T.reshape(KT, P, BS).transpose(1, 0, 2))
    alpha_t = np.ascontiguousarray(alpha).astype(bf16)
    # binary [O, G, A, QB] -> [O, G, QB, A] (+-1 exact in bf16/fp8)
    binary_t = np.ascontiguousarray(binary.transpose(0, 1, 3, 2)).astype(bin_np)
    bias = np.ascontiguousarray(bias)

    in_maps = []
    for c in range(N_CORES):
        oc, bc = divmod(c, BS_WAYS)
        osl = slice(oc * O_SH, (oc + 1) * O_SH)
        if BS_WAYS == 1:
            xc = xtp
        else:
            xc = np.ascontiguousarray(xtp[:, :, bc * BS_SH : (bc + 1) * BS_SH])
        in_maps.append(
            {
                "xt": xc,
                "alpha": alpha_t[osl],
                "binary": binary_t[osl],
                "bias": bias[osl],
            }
        )
    return in_maps


def assemble_output(results):
    out = np.empty((BS, O), dtype=np.float32)
    for c in range(N_CORES):
        oc, bc = divmod(c, BS_WAYS)
        out[
            bc * BS_SH : (bc + 1) * BS_SH, oc * O_SH : (oc + 1) * O_SH
        ] = results[c]["out"].astype(np.float32)
    return out.reshape(B, S, O)


_NC_CACHE = None


def kernel(x, alpha, bias, binary):
    global _NC_CACHE
    if _NC_CACHE is None:
        _NC_CACHE = build_nc()
    nc = _NC_CACHE
    in_maps = shard_inputs(
        np.asarray(x, dtype=np.float32),
        np.asarray(alpha, dtype=np.float32),
        np.asarray(bias, dtype=np.float32),
        np.asarray(binary, dtype=np.float32),
    )
    res = run_bass_kernel_spmd(nc, in_maps, list(range(N_CORES)))
    return assemble_output(res.results)


# revision 22
# speedup vs baseline: 1.0288x; 1.0006x over previous
"""BCQConv1D TRN2 kernel: out[b,s,o] = x[b,s,:] @ W[o,:]^T + bias[o],
W[o, g*A+a] = sum_qb alpha[o,g,qb] * binary[o,g,a,qb].

Sharding: column-parallel - alpha/binary/bias split along out_features
across the 8 NeuronCores, x replicated (each core computes the full
batch for its 512 output features).

Per core: reconstruct the W^T shard on device, keep it resident in SBUF
as bf16, then stream x^T tiles and run N=512 bf16 matmuls accumulating
over K=4096 in PSUM (s-major: one PSUM bank per 128-row output chain).
Bias is broadcast once via a K=1 ones-matmul and folded into the
PSUM->SBUF output add on the DVE. Output ships back as bf16.

Reconstruction: per (group-block, o-tile) the three bit planes are
scaled by alpha broadcast along A (step-0 AP) as [128, 8*128]
tensor_tensor ops split across DVE and GpSimd, summed on DVE, PE
transposes each [o,a] block into a dedicated 2-bank PSUM staging tile,
one [128, 512] copy per group lands the resident W^T tile.

Host side only slices/relayouts/casts inputs (x is passed
transposed/tiled [128, KT, BS] so every DMA line is contiguous).
"""

import numpy as np

import concourse.bass as bass
import concourse.tile as tile
from concourse import bacc, mybir
from concourse.bass_utils import run_bass_kernel_spmd
from concourse.masks import make_identity

# Problem shape (hardcoded per contest contract)
B, S, I, O = 4, 2048, 4096, 4096
G, A, QB = 32, 128, 3  # n_groups, group_size, qbits; G*A == I
BS = B * S  # 8192
P = 128
KT = I // P  # 32 k-tiles (== groups: i = g*A + a, A == P)

# Sharding
N_CORES = 8
O_WAYS = 8
BS_WAYS = 1
O_SH = O // O_WAYS  # per-core out features
BS_SH = BS // BS_WAYS  # per-core batch rows
NFREE = 512  # matmul moving free dim (one PSUM bank of fp32)
NB = O_SH // NFREE  # o-blocks per core
BCHUNK = 512  # bs columns fetched per x DMA
NSUB = BCHUNK // P  # matmul chains per x chunk
NGB = 8  # groups per recon DVE op block

F32 = mybir.dt.float32
BF16 = mybir.dt.bfloat16
F8E4 = mybir.dt.float8e4

OT = O_SH // P  # o-tiles for recon

BIN_FP8 = True  # ship binary as fp8e4 (half upload) vs bf16


def build_nc():
    nc = bacc.Bacc(target_bir_lowering=False)
    bin_dt = F8E4 if BIN_FP8 else BF16
    xt_d = nc.declare_dram_parameter("xt", [P, KT, BS_SH], BF16, isOutput=False)
    alpha_d = nc.declare_dram_parameter("alpha", [O_SH, G, QB], BF16, isOutput=False)
    binary_d = nc.declare_dram_parameter(
        "binary", [O_SH, G, QB, A], bin_dt, isOutput=False
    )
    bias_d = nc.declare_dram_parameter("bias", [O_SH], F32, isOutput=False)
    out_d = nc.declare_dram_parameter("out", [BS_SH, O_SH], BF16, isOutput=True)

    add = mybir.AluOpType.add
    mult = mybir.AluOpType.mult

    with tile.TileContext(nc) as tc:
        with (
            tc.tile_pool(name="const", bufs=1) as cpool,
            tc.tile_pool(name="wt", bufs=1) as wtpool,
            tc.tile_pool(name="bin16", bufs=2) as bin16p,
            tc.tile_pool(name="wacc", bufs=2) as wacc,
            tc.tile_pool(name="xp", bufs=48) as xp,
            tc.tile_pool(name="op", bufs=8) as op,
            tc.tile_pool(name="rpsum", bufs=2, space="PSUM") as rp,
            tc.tile_pool(name="psum", bufs=6, space="PSUM") as pp,
        ):
            # --- constants ---
            ident = cpool.tile([P, P], BF16, name="ident")
            make_identity(nc, ident)
            ones = cpool.tile([1, P], F32, name="ones")
            nc.vector.memset(ones, 1.0)
            bias_row = cpool.tile([1, O_SH], F32, name="bias_row")
            nc.sync.dma_start(out=bias_row, in_=bias_d.ap().unsqueeze(0))
            bias_bc = cpool.tile([P, O_SH], F32, name="bias_bc")
            for j in range(NB):
                pbt = pp.tile([P, NFREE], F32, tag="ps", name=f"psb{j}")
                nc.tensor.matmul(
                    pbt, ones, bias_row[:, j * NFREE : (j + 1) * NFREE],
                    start=True, stop=True,
                )
                nc.vector.tensor_copy(
                    out=bias_bc[:, j * NFREE : (j + 1) * NFREE], in_=pbt
                )

            # --- recon inputs: alpha resident per o-tile (fast sync ring) ---
            alpha_sb = []
            for ot in range(OT):
                at = cpool.tile([P, G, QB], BF16, name=f"alpha{ot}")
                nc.sync.dma_start(out=at, in_=alpha_d.ap()[ot * P : (ot + 1) * P])
                alpha_sb.append(at)

            # --- W^T shard, resident, one tile per k-tile (== group) ---
            wt_tiles = [
                wtpool.tile([P, O_SH], BF16, tag=f"wt{k}", name=f"wt{k}")
                for k in range(KT)
            ]

            # --- reconstruction ---
            for gp in range(G // NGB):
                g0 = gp * NGB
                planes = []
                for ot in range(OT):
                    at = alpha_sb[ot]
                    b16 = bin16p.tile([P, NGB, QB, A], bin_dt, tag=f"b16_{ot}")
                    nc.gpsimd.dma_start(
                        out=b16,
                        in_=binary_d.ap()[ot * P : (ot + 1) * P, g0 : g0 + NGB],
                    )

                    def abc(b):
                        return at[:, g0 : g0 + NGB, b, None].to_broadcast(
                            (P, NGB, A)
                        )

                    acc = wacc.tile([P, NGB, A], BF16, tag=f"sc{ot}_0")
                    tmp = wacc.tile([P, NGB, A], BF16, tag=f"sc{ot}_1")
                    nc.vector.tensor_tensor(
                        out=acc, in0=b16[:, :, 0, :], in1=abc(0), op=mult
                    )
                    nc.gpsimd.tensor_tensor(
                        out=tmp, in0=b16[:, :, 1, :], in1=abc(1), op=mult
                    )
                    nc.vector.tensor_tensor(out=acc, in0=acc, in1=tmp, op=add)
                    tmp2 = wacc.tile([P, NGB, A], BF16, tag=f"sc{ot}_1")
                    nc.vector.tensor_tensor(
                        out=tmp2, in0=b16[:, :, 2, :], in1=abc(2), op=mult
                    )
                    nc.vector.tensor_tensor(out=acc, in0=acc, in1=tmp2, op=add)
                    planes.append(acc)
                for gl in range(NGB):
                    g = g0 + gl
                    pt = rp.tile([P, O_SH], BF16, tag="rps", name=f"rps{g}")
                    for ot in range(OT):
                        nc.tensor.transpose(
                            pt[:, ot * P : (ot + 1) * P], planes[ot][:, gl, :], ident
                        )
                    nc.vector.tensor_copy(out=wt_tiles[g], in_=pt)

            # --- main matmul: out[bs, o] = x^T.T @ W^T (+bias) ---
            # s-major: each 128-row sub-chain runs its full k-loop against a
            # single PSUM bank, so chunk boundaries need only one free bank
            # at a time and drains overlap the next chain's accumulation.
            n_chunks = BS_SH // BCHUNK
            for c in range(n_chunks):
                xts = []
                for k in range(KT):
                    xt_t = xp.tile([P, BCHUNK], BF16, tag="xt")
                    dma_eng = nc.sync if k % 2 == 0 else nc.scalar
                    dma_eng.dma_start(
                        out=xt_t,
                        in_=xt_d.ap()[:, k, c * BCHUNK : (c + 1) * BCHUNK],
                    )
                    xts.append(xt_t)
                for s in range(NSUB):
                    for j in range(NB):
                        ps = pp.tile([P, NFREE], F32, tag="ps", name=f"mm{c}_{s}_{j}")
                        for k in range(KT):
                            nc.tensor.matmul(
                                ps,
                                xts[k][:, s * P : (s + 1) * P],
                                wt_tiles[k][:, j * NFREE : (j + 1) * NFREE],
                                start=(k == 0),
                                stop=(k == KT - 1),
                            )
                        os_t = op.tile([P, NFREE], BF16, tag="os")
                        nc.vector.tensor_tensor(
                            out=os_t,
                            in0=ps,
                            in1=bias_bc[:, j * NFREE : (j + 1) * NFREE],
                            op=add,
                        )
                        dma_eng = nc.sync if s % 2 == 0 else nc.scalar
                        dma_eng.dma_start(
                            out=out_d.ap()[
                                c * BCHUNK + s * P : c * BCHUNK + (s + 1) * P,
                                j * NFREE : (j + 1) * NFREE,
                            ],
                            in_=os_t,
                        )

    if not nc.is_finalized():
        nc.finalize()
    return nc


def shard_inputs(x, alpha, bias, binary):
    """Host-side slicing/relayout/dtype-cast only. Returns per-core input maps."""
    bf16 = mybir.dt.np(BF16)
    f8 = mybir.dt.np(F8E4)
    bin_np = f8 if BIN_FP8 else bf16
    x2 = np.ascontiguousarray(x).reshape(BS, I).astype(bf16)
    # xtp[p, k, s] = x2[s, k*P + p]  -> every DMA line is bs-contiguous
    xtp = np.ascontiguousarray(x2.T.reshape(KT, P, BS).transpose(1, 0, 2))
    alpha_t = np.ascontiguousarray(alpha).astype(bf16)
    # binary [O, G, A, QB] -> [O, G, QB, A] (+-1 exact in bf16/fp8)
    binary_t = np.ascontiguousarray(binary.transpose(0, 1, 3, 2)).astype(bin_np)
    bias = np.ascontiguousarray(bias)

    in_maps = []
    for c in range(N_CORES):
        oc, bc = divmod(c, BS_WAYS)
        osl = slice(oc * O_SH, (oc + 1) * O_SH)
        if BS_WAYS == 1:
            xc = xtp
        else:
            xc = np.ascontiguousarray(xtp[:, :, bc * BS_SH : (bc + 1) * BS_SH])
        in_maps.append(
            {
                "xt": xc,
                "alpha": alpha_t[osl],
                "binary": binary_t[osl],
                "bias": bias[osl],
            }
        )
    return in_maps


def assemble_output(results):
    out = np.empty((BS, O), dtype=np.float32)
    for c in range(N_CORES):
        oc, bc = divmod(c, BS_WAYS)
        out[
            bc * BS_SH : (bc + 1) * BS_SH, oc * O_SH : (oc + 1) * O_SH
        ] = results[c]["out"].astype(np.float32)
    return out.reshape(B, S, O)


_NC_CACHE = None


def kernel(x, alpha, bias, binary):
    global _NC_CACHE
    if _NC_CACHE is None:
        _NC_CACHE = build_nc()
    nc = _NC_CACHE
    in_maps = shard_inputs(
        np.asarray(x, dtype=np.float32),
        np.asarray(alpha, dtype=np.float32),
        np.asarray(bias, dtype=np.float32),
        np.asarray(binary, dtype=np.float32),
    )
    res = run_bass_kernel_spmd(nc, in_maps, list(range(N_CORES)))
    return assemble_output(res.results)


# revision 23
# speedup vs baseline: 1.0571x; 1.0275x over previous
"""BCQConv1D TRN2 kernel: out[b,s,o] = x[b,s,:] @ W[o,:]^T + bias[o],
W[o, g*A+a] = sum_qb alpha[o,g,qb] * binary[o,g,a,qb].

Sharding: column-parallel - alpha/binary/bias split along out_features
across the 8 NeuronCores, x replicated (each core computes the full
batch for its 512 output features).

Per core: reconstruct the W^T shard on device, keep it resident in SBUF
as bf16, then stream x^T tiles and run N=512 bf16 matmuls accumulating
over K=4096 in PSUM (s-major: one PSUM bank per 128-row output chain).
Bias is broadcast once via a K=1 ones-matmul and folded into the
PSUM->SBUF output add on the DVE. Output ships back as bf16.

Reconstruction: per (group-block, o-tile) the three bit planes are
scaled by alpha broadcast along A (step-0 AP) as [128, 8*128]
tensor_tensor ops split across DVE and GpSimd, summed on DVE, PE
transposes each [o,a] block into a dedicated 2-bank PSUM staging tile,
one [128, 512] copy per group lands the resident W^T tile.

Host side only slices/relayouts/casts inputs (x is passed
transposed/tiled [128, KT, BS] so every DMA line is contiguous).
"""

import numpy as np

import concourse.bass as bass
import concourse.tile as tile
from concourse import bacc, mybir
from concourse.bass_utils import run_bass_kernel_spmd
from concourse.masks import make_identity

# Problem shape (hardcoded per contest contract)
B, S, I, O = 4, 2048, 4096, 4096
G, A, QB = 32, 128, 3  # n_groups, group_size, qbits; G*A == I
BS = B * S  # 8192
P = 128
KT = I // P  # 32 k-tiles (== groups: i = g*A + a, A == P)

# Sharding
N_CORES = 8
O_WAYS = 8
BS_WAYS = 1
O_SH = O // O_WAYS  # per-core out features
BS_SH = BS // BS_WAYS  # per-core batch rows
NFREE = 512  # matmul moving free dim (one PSUM bank of fp32)
NB = O_SH // NFREE  # o-blocks per core
BCHUNK = 512  # bs columns fetched per x DMA
NSUB = BCHUNK // P  # matmul chains per x chunk
NGB = 8  # groups per recon DVE op block

F32 = mybir.dt.float32
BF16 = mybir.dt.bfloat16
F8E4 = mybir.dt.float8e4

OT = O_SH // P  # o-tiles for recon

BIN_FP8 = True  # ship binary as fp8e4 (half upload) vs bf16


def build_nc():
    nc = bacc.Bacc(target_bir_lowering=False)
    bin_dt = F8E4 if BIN_FP8 else BF16
    xt_d = nc.declare_dram_parameter("xt", [P, KT, BS_SH], BF16, isOutput=False)
    alpha_d = nc.declare_dram_parameter("alpha", [O_SH, G, QB], BF16, isOutput=False)
    binary_d = nc.declare_dram_parameter(
        "binary", [O_SH, G, QB, A], bin_dt, isOutput=False
    )
    bias_d = nc.declare_dram_parameter("bias", [O_SH], F32, isOutput=False)
    out_d = nc.declare_dram_parameter("out", [BS_SH, O_SH], BF16, isOutput=True)

    add = mybir.AluOpType.add
    mult = mybir.AluOpType.mult

    with tile.TileContext(nc) as tc:
        with (
            tc.tile_pool(name="const", bufs=1) as cpool,
            tc.tile_pool(name="wt", bufs=1) as wtpool,
            tc.tile_pool(name="bin16", bufs=2) as bin16p,
            tc.tile_pool(name="wacc", bufs=2) as wacc,
            tc.tile_pool(name="xp", bufs=64) as xp,
            tc.tile_pool(name="op", bufs=8) as op,
            tc.tile_pool(name="rpsum", bufs=1, space="PSUM") as rp,
            tc.tile_pool(name="psum", bufs=7, space="PSUM") as pp,
        ):
            # --- constants ---
            ident = cpool.tile([P, P], BF16, name="ident")
            make_identity(nc, ident)
            ones = cpool.tile([1, P], F32, name="ones")
            nc.vector.memset(ones, 1.0)
            bias_row = cpool.tile([1, O_SH], F32, name="bias_row")
            nc.sync.dma_start(out=bias_row, in_=bias_d.ap().unsqueeze(0))
            bias_bc = cpool.tile([P, O_SH], F32, name="bias_bc")
            for j in range(NB):
                pbt = pp.tile([P, NFREE], F32, tag="ps", name=f"psb{j}")
                nc.tensor.matmul(
                    pbt, ones, bias_row[:, j * NFREE : (j + 1) * NFREE],
                    start=True, stop=True,
                )
                nc.vector.tensor_copy(
                    out=bias_bc[:, j * NFREE : (j + 1) * NFREE], in_=pbt
                )

            # --- recon inputs: alpha resident per o-tile (fast sync ring) ---
            alpha_sb = []
            for ot in range(OT):
                at = cpool.tile([P, G, QB], BF16, name=f"alpha{ot}")
                nc.sync.dma_start(out=at, in_=alpha_d.ap()[ot * P : (ot + 1) * P])
                alpha_sb.append(at)

            # --- W^T shard, resident, one tile per k-tile (== group) ---
            wt_tiles = [
                wtpool.tile([P, O_SH], BF16, tag=f"wt{k}", name=f"wt{k}")
                for k in range(KT)
            ]

            # --- reconstruction ---
            for gp in range(G // NGB):
                g0 = gp * NGB
                planes = []
                for ot in range(OT):
                    at = alpha_sb[ot]
                    b16 = bin16p.tile([P, NGB, QB, A], bin_dt, tag=f"b16_{ot}")
                    nc.gpsimd.dma_start(
                        out=b16,
                        in_=binary_d.ap()[ot * P : (ot + 1) * P, g0 : g0 + NGB],
                    )

                    def abc(b):
                        return at[:, g0 : g0 + NGB, b, None].to_broadcast(
                            (P, NGB, A)
                        )

                    acc = wacc.tile([P, NGB, A], BF16, tag=f"sc{ot}_0")
                    tmp = wacc.tile([P, NGB, A], BF16, tag=f"sc{ot}_1")
                    nc.vector.tensor_tensor(
                        out=acc, in0=b16[:, :, 0, :], in1=abc(0), op=mult
                    )
                    nc.gpsimd.tensor_tensor(
                        out=tmp, in0=b16[:, :, 1, :], in1=abc(1), op=mult
                    )
                    nc.vector.tensor_tensor(out=acc, in0=acc, in1=tmp, op=add)
                    tmp2 = wacc.tile([P, NGB, A], BF16, tag=f"sc{ot}_1")
                    nc.vector.tensor_tensor(
                        out=tmp2, in0=b16[:, :, 2, :], in1=abc(2), op=mult
                    )
                    nc.vector.tensor_tensor(out=acc, in0=acc, in1=tmp2, op=add)
                    planes.append(acc)
                for gl in range(NGB):
                    g = g0 + gl
                    pt = rp.tile([P, O_SH], BF16, tag="rps", name=f"rps{g}")
                    for ot in range(OT):
                        nc.tensor.transpose(
                            pt[:, ot * P : (ot + 1) * P], planes[ot][:, gl, :], ident
                        )
                    nc.vector.tensor_copy(out=wt_tiles[g], in_=pt)

            # --- main matmul: out[bs, o] = x^T.T @ W^T (+bias) ---
            # s-major: each 128-row sub-chain runs its full k-loop against a
            # single PSUM bank, so chunk boundaries need only one free bank
            # at a time and drains overlap the next chain's accumulation.
            n_chunks = BS_SH // BCHUNK
            for c in range(n_chunks):
                xts = []
                for k in range(KT):
                    xt_t = xp.tile([P, BCHUNK], BF16, tag="xt")
                    dma_eng = nc.sync if k % 2 == 0 else nc.scalar
                    dma_eng.dma_start(
                        out=xt_t,
                        in_=xt_d.ap()[:, k, c * BCHUNK : (c + 1) * BCHUNK],
                    )
                    xts.append(xt_t)
                for s in range(NSUB):
                    for j in range(NB):
                        ps = pp.tile([P, NFREE], F32, tag="ps", name=f"mm{c}_{s}_{j}")
                        for k in range(KT):
                            nc.tensor.matmul(
                                ps,
                                xts[k][:, s * P : (s + 1) * P],
                                wt_tiles[k][:, j * NFREE : (j + 1) * NFREE],
                                start=(k == 0),
                                stop=(k == KT - 1),
                            )
                        os_t = op.tile([P, NFREE], BF16, tag="os")
                        nc.vector.tensor_tensor(
                            out=os_t,
                            in0=ps,
                            in1=bias_bc[:, j * NFREE : (j + 1) * NFREE],
                            op=add,
                        )
                        dma_eng = nc.sync if s % 2 == 0 else nc.scalar
                        dma_eng.dma_start(
                            out=out_d.ap()[
                                c * BCHUNK + s * P : c * BCHUNK + (s + 1) * P,
                                j * NFREE : (j + 1) * NFREE,
                            ],
                            in_=os_t,
                        )

    if not nc.is_finalized():
        nc.finalize()
    return nc


def shard_inputs(x, alpha, bias, binary):
    """Host-side slicing/relayout/dtype-cast only. Returns per-core input maps."""
    bf16 = mybir.dt.np(BF16)
    f8 = mybir.dt.np(F8E4)
    bin_np = f8 if BIN_FP8 else bf16
    x2 = np.ascontiguousarray(x).reshape(BS, I).astype(bf16)
    # xtp[p, k, s] = x2[s, k*P + p]  -> every DMA line is bs-contiguous
    xtp = np.ascontiguousarray(x2.T.reshape(KT, P, BS).transpose(1, 0, 2))
    alpha_t = np.ascontiguousarray(alpha).astype(bf16)
    # binary [O, G, A, QB] -> [O, G, QB, A] (+-1 exact in bf16/fp8)
    binary_t = np.ascontiguousarray(binary.transpose(0, 1, 3, 2)).astype(bin_np)
    bias = np.ascontiguousarray(bias)

    in_maps = []
    for c in range(N_CORES):
        oc, bc = divmod(c, BS_WAYS)
        osl = slice(oc * O_SH, (oc + 1) * O_SH)
        if BS_WAYS == 1:
            xc = xtp
        else:
            xc = np.ascontiguousarray(xtp[:, :, bc * BS_SH : (bc + 1) * BS_SH])
        in_maps.append(
            {
                "xt": xc,
                "alpha": alpha_t[osl],
                "binary": binary_t[osl],
                "bias": bias[osl],
            }
        )
    return in_maps


def assemble_output(results):
    out = np.empty((BS, O), dtype=np.float32)
    for c in range(N_CORES):
        oc, bc = divmod(c, BS_WAYS)
        out[
            bc * BS_SH : (bc + 1) * BS_SH, oc * O_SH : (oc + 1) * O_SH
        ] = results[c]["out"].astype(np.float32)
    return out.reshape(B, S, O)


_NC_CACHE = None


def kernel(x, alpha, bias, binary):
    global _NC_CACHE
    if _NC_CACHE is None:
        _NC_CACHE = build_nc()
    nc = _NC_CACHE
    in_maps = shard_inputs(
        np.asarray(x, dtype=np.float32),
        np.asarray(alpha, dtype=np.float32),
        np.asarray(bias, dtype=np.float32),
        np.asarray(binary, dtype=np.float32),
    )
    res = run_bass_kernel_spmd(nc, in_maps, list(range(N_CORES)))
    return assemble_output(res.results)


# revision 24
# speedup vs baseline: 1.0603x; 1.0030x over previous
"""BCQConv1D TRN2 kernel: out[b,s,o] = x[b,s,:] @ W[o,:]^T + bias[o],
W[o, g*A+a] = sum_qb alpha[o,g,qb] * binary[o,g,a,qb].

Sharding: column-parallel - alpha/binary/bias split along out_features
across the 8 NeuronCores, x replicated (each core computes the full
batch for its 512 output features).

Per core: reconstruct the W^T shard on device, keep it resident in SBUF
as bf16, then stream x^T tiles and run N=512 bf16 matmuls accumulating
over K=4096 in PSUM (s-major: one PSUM bank per 128-row output chain).
Bias is broadcast once via a K=1 ones-matmul and folded into the
PSUM->SBUF output add on the DVE. Output ships back as bf16.

Reconstruction: per (group-block, o-tile) the three bit planes are
scaled by alpha broadcast along A (step-0 AP) as [128, 8*128]
tensor_tensor ops split across DVE and GpSimd, summed on DVE, PE
transposes each [o,a] block into a dedicated 2-bank PSUM staging tile,
one [128, 512] copy per group lands the resident W^T tile.

Host side only slices/relayouts/casts inputs (x is passed
transposed/tiled [128, KT, BS] so every DMA line is contiguous).
"""

import numpy as np

import concourse.bass as bass
import concourse.tile as tile
from concourse import bacc, mybir
from concourse.bass_utils import run_bass_kernel_spmd
from concourse.masks import make_identity

# Problem shape (hardcoded per contest contract)
B, S, I, O = 4, 2048, 4096, 4096
G, A, QB = 32, 128, 3  # n_groups, group_size, qbits; G*A == I
BS = B * S  # 8192
P = 128
KT = I // P  # 32 k-tiles (== groups: i = g*A + a, A == P)

# Sharding
N_CORES = 8
O_WAYS = 8
BS_WAYS = 1
O_SH = O // O_WAYS  # per-core out features
BS_SH = BS // BS_WAYS  # per-core batch rows
NFREE = 512  # matmul moving free dim (one PSUM bank of fp32)
NB = O_SH // NFREE  # o-blocks per core
BCHUNK = 512  # bs columns fetched per x DMA
NSUB = BCHUNK // P  # matmul chains per x chunk
NGB = 8  # groups per recon DVE op block

F32 = mybir.dt.float32
BF16 = mybir.dt.bfloat16
F8E4 = mybir.dt.float8e4

OT = O_SH // P  # o-tiles for recon

BIN_FP8 = True  # ship binary as fp8e4 (half upload) vs bf16


def build_nc():
    nc = bacc.Bacc(target_bir_lowering=False)
    bin_dt = F8E4 if BIN_FP8 else BF16
    xt_d = nc.declare_dram_parameter("xt", [P, KT, BS_SH], BF16, isOutput=False)
    alpha_d = nc.declare_dram_parameter("alpha", [O_SH, G, QB], BF16, isOutput=False)
    binary_d = nc.declare_dram_parameter(
        "binary", [O_SH, G, QB, A], bin_dt, isOutput=False
    )
    bias_d = nc.declare_dram_parameter("bias", [O_SH], F32, isOutput=False)
    out_d = nc.declare_dram_parameter("out", [BS_SH, O_SH], BF16, isOutput=True)

    add = mybir.AluOpType.add
    mult = mybir.AluOpType.mult

    with tile.TileContext(nc) as tc:
        with (
            tc.tile_pool(name="const", bufs=1) as cpool,
            tc.tile_pool(name="wt", bufs=1) as wtpool,
            tc.tile_pool(name="bin16", bufs=2) as bin16p,
            tc.tile_pool(name="wacc", bufs=2) as wacc,
            tc.tile_pool(name="xp", bufs=64) as xp,
            tc.tile_pool(name="op", bufs=8) as op,
            tc.tile_pool(name="rpsum", bufs=1, space="PSUM") as rp,
            tc.tile_pool(name="psum", bufs=7, space="PSUM") as pp,
        ):
            # --- constants ---
            ident = cpool.tile([P, P], BF16, name="ident")
            make_identity(nc, ident)
            ones = cpool.tile([1, P], F32, name="ones")
            nc.vector.memset(ones, 1.0)
            bias_row = cpool.tile([1, O_SH], F32, name="bias_row")
            nc.sync.dma_start(out=bias_row, in_=bias_d.ap().unsqueeze(0))
            bias_bc = cpool.tile([P, O_SH], F32, name="bias_bc")
            for j in range(NB):
                pbt = pp.tile([P, NFREE], F32, tag="ps", name=f"psb{j}")
                nc.tensor.matmul(
                    pbt, ones, bias_row[:, j * NFREE : (j + 1) * NFREE],
                    start=True, stop=True,
                )
                nc.vector.tensor_copy(
                    out=bias_bc[:, j * NFREE : (j + 1) * NFREE], in_=pbt
                )

            # --- recon inputs: alpha resident per o-tile (fast sync ring) ---
            alpha_sb = []
            for ot in range(OT):
                at = cpool.tile([P, G, QB], BF16, name=f"alpha{ot}")
                nc.sync.dma_start(out=at, in_=alpha_d.ap()[ot * P : (ot + 1) * P])
                alpha_sb.append(at)

            # --- W^T shard, resident, one tile per k-tile (== group) ---
            wt_tiles = [
                wtpool.tile([P, O_SH], BF16, tag=f"wt{k}", name=f"wt{k}")
                for k in range(KT)
            ]

            # --- reconstruction ---
            for gp in range(G // NGB):
                g0 = gp * NGB
                planes = []
                for ot in range(OT):
                    at = alpha_sb[ot]
                    b16 = bin16p.tile([P, NGB, QB, A], bin_dt, tag=f"b16_{ot}")
                    nc.gpsimd.dma_start(
                        out=b16,
                        in_=binary_d.ap()[ot * P : (ot + 1) * P, g0 : g0 + NGB],
                    )

                    def abc(b):
                        return at[:, g0 : g0 + NGB, b, None].to_broadcast(
                            (P, NGB, A)
                        )

                    acc = wacc.tile([P, NGB, A], BF16, tag=f"sc{ot}_0")
                    tmp = wacc.tile([P, NGB, A], BF16, tag=f"sc{ot}_1")
                    nc.vector.tensor_tensor(
                        out=acc, in0=b16[:, :, 0, :], in1=abc(0), op=mult
                    )
                    nc.gpsimd.tensor_tensor(
                        out=tmp, in0=b16[:, :, 1, :], in1=abc(1), op=mult
                    )
                    nc.vector.tensor_tensor(out=acc, in0=acc, in1=tmp, op=add)
                    tmp2 = wacc.tile([P, NGB, A], BF16, tag=f"sc{ot}_1")
                    nc.vector.tensor_tensor(
                        out=tmp2, in0=b16[:, :, 2, :], in1=abc(2), op=mult
                    )
                    nc.vector.tensor_tensor(out=acc, in0=acc, in1=tmp2, op=add)
                    planes.append(acc)
                for gl in range(NGB):
                    g = g0 + gl
                    pt = rp.tile([P, O_SH], BF16, tag="rps", name=f"rps{g}")
                    for ot in range(OT):
                        nc.tensor.transpose(
                            pt[:, ot * P : (ot + 1) * P], planes[ot][:, gl, :], ident
                        )
                    nc.scalar.copy(out=wt_tiles[g], in_=pt)

            # --- main matmul: out[bs, o] = x^T.T @ W^T (+bias) ---
            # s-major: each 128-row sub-chain runs its full k-loop against a
            # single PSUM bank, so chunk boundaries need only one free bank
            # at a time and drains overlap the next chain's accumulation.
            n_chunks = BS_SH // BCHUNK
            for c in range(n_chunks):
                xts = []
                for k in range(KT):
                    xt_t = xp.tile([P, BCHUNK], BF16, tag="xt")
                    dma_eng = nc.sync if k % 2 == 0 else nc.scalar
                    dma_eng.dma_start(
                        out=xt_t,
                        in_=xt_d.ap()[:, k, c * BCHUNK : (c + 1) * BCHUNK],
                    )
                    xts.append(xt_t)
                for s in range(NSUB):
                    for j in range(NB):
                        ps = pp.tile([P, NFREE], F32, tag="ps", name=f"mm{c}_{s}_{j}")
                        for k in range(KT):
                            nc.tensor.matmul(
                                ps,
                                xts[k][:, s * P : (s + 1) * P],
                                wt_tiles[k][:, j * NFREE : (j + 1) * NFREE],
                                start=(k == 0),
                                stop=(k == KT - 1),
                            )
                        os_t = op.tile([P, NFREE], BF16, tag="os")
                        nc.vector.tensor_tensor(
                            out=os_t,
                            in0=ps,
                            in1=bias_bc[:, j * NFREE : (j + 1) * NFREE],
                            op=add,
                        )
                        dma_eng = nc.sync if s % 2 == 0 else nc.scalar
                        dma_eng.dma_start(
                            out=out_d.ap()[
                                c * BCHUNK + s * P : c * BCHUNK + (s + 1) * P,
                                j * NFREE : (j + 1) * NFREE,
                            ],
                            in_=os_t,
                        )

    if not nc.is_finalized():
        nc.finalize()
    return nc


def shard_inputs(x, alpha, bias, binary):
    """Host-side slicing/relayout/dtype-cast only. Returns per-core input maps."""
    bf16 = mybir.dt.np(BF16)
    f8 = mybir.dt.np(F8E4)
    bin_np = f8 if BIN_FP8 else bf16
    x2 = np.ascontiguousarray(x).reshape(BS, I).astype(bf16)
    # xtp[p, k, s] = x2[s, k*P + p]  -> every DMA line is bs-contiguous
    xtp = np.ascontiguousarray(x2.T.reshape(KT, P, BS).transpose(1, 0, 2))
    alpha_t = np.ascontiguousarray(alpha).astype(bf16)
    # binary [O, G, A, QB] -> [O, G, QB, A] (+-1 exact in bf16/fp8)
    binary_t = np.ascontiguousarray(binary.transpose(0, 1, 3, 2)).astype(bin_np)
    bias = np.ascontiguousarray(bias)

    in_maps = []
    for c in range(N_CORES):
        oc, bc = divmod(c, BS_WAYS)
        osl = slice(oc * O_SH, (oc + 1) * O_SH)
        if BS_WAYS == 1:
            xc = xtp
        else:
            xc = np.ascontiguousarray(xtp[:, :, bc * BS_SH : (bc + 1) * BS_SH])
        in_maps.append(
            {
                "xt": xc,
                "alpha": alpha_t[osl],
                "binary": binary_t[osl],
                "bias": bias[osl],
            }
        )
    return in_maps


def assemble_output(results):
    out = np.empty((BS, O), dtype=np.float32)
    for c in range(N_CORES):
        oc, bc = divmod(c, BS_WAYS)
        out[
            bc * BS_SH : (bc + 1) * BS_SH, oc * O_SH : (oc + 1) * O_SH
        ] = results[c]["out"].astype(np.float32)
    return out.reshape(B, S, O)


_NC_CACHE = None


def kernel(x, alpha, bias, binary):
    global _NC_CACHE
    if _NC_CACHE is None:
        _NC_CACHE = build_nc()
    nc = _NC_CACHE
    in_maps = shard_inputs(
        np.asarray(x, dtype=np.float32),
        np.asarray(alpha, dtype=np.float32),
        np.asarray(bias, dtype=np.float32),
        np.asarray(binary, dtype=np.float32),
    )
    res = run_bass_kernel_spmd(nc, in_maps, list(range(N_CORES)))
    return assemble_output(res.results)
